# revision 11
# baseline (speedup 1.0000x reference)
"""Trainium2 Bass kernel for the 6-layer bidirectional GRU problem.

Key structural insight: the reference consumes only layer_input[..., :128]
(= first 128 features of the FORWARD output) as the next layer's input, so
the backward scans of layers 0-4 are dead code.  Only 7 GRU scans matter:
  scan 0..5: forward scans of layers 0..5 (sequentially chained)
  scan 6:    backward scan of layer 5 (input = fwd layer-4 output, reversed)

Layout ("gates transposed"): gate dim (768 = 6 chunks of 128) on SBUF
partitions, (timestep, batch) on the free dim.  Hidden state history for a
scan is a (128, S, 2, B) tile: [p, t, k, b] holds h[t, b, k*128+p].

Per step: 12 fp32 matmuls (weight tiles (128,128) stationary, h columns
moving, N=B=10) accumulate gh.T into PSUM (128, 6, B); input-side gate
contributions gx are precomputed per window of CW steps with big matmuls.
Elementwise: DVE adds/muls + ACT sigmoid/tanh produce h_t.
"""

import os
import tempfile

import numpy as np

import concourse.bass as bass
import concourse.tile as tile
from concourse import bacc, mybir
from concourse.bass_utils import run_bass_kernel_spmd

S = 512
B = 10
I = 128
H = 256
NL = 6
NSCAN = 7
CW = 64  # window size (steps) for bulk gx precompute
SUB = 32  # sub-window (steps) per bulk matmul: N = SUB*B = 320 <= 512
F32 = mybir.dt.float32

_BUILD_CACHE: dict = {}


def _build(nscan=NSCAN, steps=S):
    nc = bacc.Bacc("TRN2", target_bir_lowering=False, debug=False, num_devices=1)
    xT_d = nc.dram_tensor("xT", (128, steps, B), F32, kind="ExternalInput")
    whhT_d = nc.dram_tensor("whhT", (nscan, 128, 12, 128), F32, kind="ExternalInput")
    wihT_d = nc.dram_tensor("wihT", (nscan, 128, 6, 128), F32, kind="ExternalInput")
    # outputs stay in the on-chip hist layout [p, t, k, b]; host transposes
    outF_d = nc.dram_tensor("outF", (128, steps, 2, B), F32, kind="ExternalOutput")
    outB_d = nc.dram_tensor("outB", (128, steps, 2, B), F32, kind="ExternalOutput")

    nwin = steps // CW
    SIG = mybir.ActivationFunctionType.Sigmoid
    TANH = mybir.ActivationFunctionType.Tanh

    with tile.TileContext(nc) as tc:
        with (
            tc.tile_pool(name="hist", bufs=1) as hist_pool,
            tc.tile_pool(name="xin", bufs=1) as xin_pool,
            tc.tile_pool(name="wts", bufs=2) as wts_pool,
            tc.tile_pool(name="gx", bufs=2) as gx_pool,
            tc.tile_pool(name="small", bufs=4) as small,
            tc.tile_pool(name="psg", bufs=2, space="PSUM") as psg_pool,
            tc.tile_pool(name="psx", bufs=2, space="PSUM") as psx_pool,
        ):
            histA = hist_pool.tile([128, steps, 2, B], F32, tag="histA")
            histB = hist_pool.tile([128, steps, 2, B], F32, tag="histB")
            xT = xin_pool.tile([128, steps, B], F32, tag="xT")
            zero20 = xin_pool.tile([128, 2, B], F32, tag="zero")

            nc.sync.dma_start(xT[:], xT_d[:])
            nc.vector.memset(zero20[:], 0.0)

            for s in range(nscan):
                is_bwd = s == nscan - 1 and nscan == NSCAN
                if is_bwd:
                    # bwd scan reads f4's history (histA, s=4 parity) and may
                    # overwrite histB (f5's history, already streamed out).
                    hist_prev, hist_cur = histA, histB
                else:
                    hist_cur = histA if s % 2 == 0 else histB
                    hist_prev = histB if s % 2 == 0 else histA

                wh = wts_pool.tile([128, 12, 128], F32, tag="wh")
                wi = wts_pool.tile([128, 6, 128], F32, tag="wi")
                nc.sync.dma_start(wh[:], whhT_d[s])
                nc.sync.dma_start(wi[:], wihT_d[s])

                for w in range(nwin):
                    # ---- bulk gx for this window: gx[c] = W_ih[c] @ xin ----
                    # one PSUM tile per sub-matmul: a matmul output must not
                    # cross a PSUM bank boundary (SUB*B*4 = 1280 <= 2048).
                    gx = gx_pool.tile([128, 6, CW, B], F32, tag="gx")
                    for c in range(6):
                        for q in range(CW // SUB):
                            psx = psx_pool.tile([128, SUB, B], F32, tag="psx")
                            if s == 0:
                                t0 = w * CW + q * SUB
                                rhs = xT[:, t0 : t0 + SUB, :]
                            elif not is_bwd:
                                t0 = w * CW + q * SUB
                                rhs = hist_prev[:, t0 : t0 + SUB, 0, :]
                            else:
                                # scan-step j consumes t = S-1-j; window w
                                # needs t in [S-(w+1)*CW, S-w*CW), stored
                                # ascending; per-step index mirrors inside.
                                t0 = steps - (w + 1) * CW + q * SUB
                                rhs = hist_prev[:, t0 : t0 + SUB, 0, :]
                            nc.tensor.matmul(
                                psx[:],
                                wi[:, c, :],
                                rhs,
                                start=True,
                                stop=True,
                            )
                            nc.scalar.copy(
                                gx[:, c, q * SUB : (q + 1) * SUB, :], psx[:]
                            )

                    # ---- sequential steps in this window ----
                    for j_loc in range(CW):
                        j = w * CW + j_loc
                        # gx "column" for this step: for bwd, the bulk above
                        # stored ascending t; step j_loc needs t=S-1-j which
                        # sits at reversed position inside the window.
                        g = j_loc if not is_bwd else (CW - 1 - j_loc)
                        psg = psg_pool.tile([128, 6, B], F32, tag="psg")
                        if j == 0:
                            hprev = zero20[:]
                        else:
                            hprev = hist_cur[:, j - 1]
                        for c in range(6):
                            for k in range(2):
                                nc.tensor.matmul(
                                    psg[:, c, :],
                                    wh[:, c * 2 + k, :],
                                    hprev[:, k, :],
                                    start=(k == 0),
                                    stop=(k == 1),
                                )
                        rzsum = small.tile([128, 4, B], F32, tag="rzsum")
                        nc.vector.tensor_add(
                            rzsum[:], psg[:, 0:4, :], gx[:, 0:4, g, :]
                        )
                        rz = small.tile([128, 4, B], F32, tag="rz")
                        nc.scalar.activation(rz[:], rzsum[:], SIG)
                        t1 = small.tile([128, 2, B], F32, tag="t1")
                        nc.vector.tensor_mul(t1[:], rz[:, 0:2, :], psg[:, 4:6, :])
                        t2 = small.tile([128, 2, B], F32, tag="t2")
                        nc.vector.tensor_add(t2[:], t1[:], gx[:, 4:6, g, :])
                        n_t = small.tile([128, 2, B], F32, tag="n")
                        nc.scalar.activation(n_t[:], t2[:], TANH)
                        d = small.tile([128, 2, B], F32, tag="d")
                        nc.vector.tensor_sub(d[:], hprev, n_t[:])
                        e = small.tile([128, 2, B], F32, tag="e")
                        nc.vector.tensor_mul(e[:], rz[:, 2:4, :], d[:])
                        nc.vector.tensor_add(hist_cur[:, j], e[:], n_t[:])

                    # ---- stream window's h to DRAM for the two output scans
                    if s in (nscan - 2, nscan - 1):
                        dst_d = outF_d if s == nscan - 2 else outB_d
                        nc.sync.dma_start(
                            dst_d[:, bass.ts(w, CW)],
                            hist_cur[:, bass.ts(w, CW)],
                        )
    nc.compile()
    return nc


def _pack_inputs(x, w_ih_l0, w_ih_rest, w_hh):
    x = np.asarray(x, dtype=np.float32)
    w_ih_l0 = np.asarray(w_ih_l0, dtype=np.float32)
    w_ih_rest = np.asarray(w_ih_rest, dtype=np.float32)
    w_hh = np.asarray(w_hh, dtype=np.float32)

    whhT = np.empty((NSCAN, 128, 12, 128), dtype=np.float32)
    wihT = np.empty((NSCAN, 128, 6, 128), dtype=np.float32)
    for s in range(NSCAN):
        if s < NL:
            whs = w_hh[s, 0]
            wis = w_ih_l0[0][:, :I] if s == 0 else w_ih_rest[s - 1, 0][:, :I]
        else:
            whs = w_hh[NL - 1, 1]
            wis = w_ih_rest[NL - 2, 1][:, :I]
        for c in range(6):
            for k in range(2):
                whhT[s, :, c * 2 + k, :] = whs[
                    c * 128 : (c + 1) * 128, k * 128 : (k + 1) * 128
                ].T
            wihT[s, :, c, :] = wis[c * 128 : (c + 1) * 128, :].T
    xT = np.ascontiguousarray(x.reshape(S * B, I).T.reshape(128, S, B))
    return {"xT": xT, "whhT": whhT, "wihT": wihT}


def kernel(x, w_ih_l0, w_ih_rest, w_hh):
    in_map = _pack_inputs(x, w_ih_l0, w_ih_rest, w_hh)
    if "nc" not in _BUILD_CACHE:
        _BUILD_CACHE["nc"] = _build()
    nc = _BUILD_CACHE["nc"]
    trace = bool(int(os.environ.get("GRU_TRACE", "0")))
    tmpdir = os.environ.get("GRU_TMPDIR") or tempfile.mkdtemp()
    res = run_bass_kernel_spmd(nc, [in_map], core_ids=[0], trace=trace, tmpdir=tmpdir)
    _BUILD_CACHE["last_result"] = res
    r = res.results[0]
    # [p, t, k, b] -> [t, b, k*128+p]
    outF = r["outF"].reshape(128, S, 2, B).transpose(1, 3, 2, 0).reshape(S, B, 256)
    outB = r["outB"].reshape(128, S, 2, B).transpose(1, 3, 2, 0).reshape(S, B, 256)
    return np.concatenate([outF, outB], axis=-1).astype(np.float32)
